# revision 6
# baseline (speedup 1.0000x reference)
"""NT-Xent loss on 8 Trainium2 cores (v2: fp8 DoubleRow matmuls).

Math: with row-normalized views zjn, zin and r = [zjn; zin],
S = r@r.T / T, pos_i = (zjn_i . zin_i)/T, the kept logits for row i are
the same-view off-diagonal entries plus pos_i.  Since all cosine logits
are <= 1/T = 10 (diagonal exactly 10), use the fixed shift 10:

  lse_i  = 10 + ln( rowsum_i - diag_i + epos_i )
  loss   = mean(lse_i - pos_i)

where rowsum_i = sum_j exp(S_same[i,j] - 10) over the FULL same-view Gram
row (diagonal included), diag_i = the device's own diagonal term
(emulated exactly on host from the fp8 operands incl. bf16 rounding),
epos_i = exp(pos_i - 10).

Device work per core (SPMD, cores 0-3 view zj, cores 4-7 view zi; each
owns a 1024-row slab): rows are prescaled by 16 and quantized to
fp8e4m3; G = qnT.T @ anT via DoubleRow matmuls (K=256 in one pass),
ACT exp(G*(10/256) - 10) -> bf16, DVE row-sum reduce.  Everything
O(N*D) or smaller (normalize, pos, final log/mean) runs on host.
"""

import numpy as np
import ml_dtypes

N = 4096
D = 256
TEMP = 0.1
NCORES = 8
RPC = 2 * N // NCORES          # 1024 rows per core
IT = RPC // 128                # 8 i-tiles of 128 rows
HALF = 2048                    # j-chunk per PSUM buffer / ACT op
NH = N // HALF                 # 2 halves of the 4096-wide Gram row
NCH = HALF // 512              # 4 column chunks per half
SC = 16.0                      # fp8 prescale (power of 2, exact)
ASCALE = (1.0 / TEMP) / (SC * SC)   # 10/256 applied in ACT

_CACHE = {}


def _build_program():
    if "nc" in _CACHE:
        return _CACHE["nc"]

    import concourse.bass as bass
    import concourse.tile as tile
    from concourse import bacc, mybir

    F8 = mybir.dt.float8e4
    BF16 = mybir.dt.bfloat16
    F32 = mybir.dt.float32

    nc = bacc.Bacc(
        "TRN2", target_bir_lowering=False, debug=False, num_devices=NCORES
    )

    # anT[h][c][p][k][col] = a8[h*2048 + c*512 + col, k*128 + p]
    anT_d = nc.dram_tensor("anT", [NH, NCH, 128, 2, 512], F8, kind="ExternalInput")
    # qnT[p][k][r] = q8slab[r, k*128 + p]
    qnT_d = nc.dram_tensor("qnT", [128, 2, RPC], F8, kind="ExternalInput")
    rs_d = nc.dram_tensor("rs", [128, IT], F32, kind="ExternalOutput")

    with tile.TileContext(nc) as tc:
        with (
            tc.tile_pool(name="weights", bufs=1) as wpool,
            tc.tile_pool(name="scratch", bufs=2) as spool,
            tc.tile_pool(name="psum", bufs=2, space="PSUM") as ppool,
        ):
            qnT = wpool.tile([128, 2, RPC], F8)
            nc.sync.dma_start(out=qnT[:], in_=qnT_d[:])
            an = [
                [wpool.tile([128, 2, 512], F8, name=f"an{h}_{c}") for c in range(NCH)]
                for h in range(NH)
            ]
            # h=0 chunks on the sync queue (needed first), h=1 on gpsimd
            for c in range(NCH):
                nc.sync.dma_start(out=an[0][c][:], in_=anT_d[0, c])
            for c in range(NCH):
                nc.gpsimd.dma_start(out=an[1][c][:], in_=anT_d[1, c])

            acc = wpool.tile([128, IT, NH], F32)
            rs = wpool.tile([128, IT], F32)
            bias = wpool.tile([128, 1], F32)
            nc.vector.memset(bias[:], -1.0 / TEMP)

            for h in range(NH):
                for t in range(IT):
                    ps = ppool.tile([128, HALF], F32)
                    for c in range(NCH):
                        nc.tensor.matmul(
                            ps[:, c * 512:(c + 1) * 512],
                            qnT[:, :, t * 128:(t + 1) * 128],
                            an[h][c][:],
                            start=True,
                            stop=True,
                            perf_mode=mybir.MatmulPerfMode.DoubleRow,
                        )
                    sc = spool.tile([128, HALF], BF16)
                    nc.scalar.activation(
                        sc[:],
                        ps[:],
                        mybir.ActivationFunctionType.Exp,
                        bias=bias[:],
                        scale=ASCALE,
                    )
                    nc.vector.tensor_reduce(
                        acc[:, t, h:h + 1],
                        sc[:],
                        axis=mybir.AxisListType.X,
                        op=mybir.AluOpType.add,
                    )

            nc.vector.tensor_reduce(
                rs[:], acc[:], axis=mybir.AxisListType.X, op=mybir.AluOpType.add
            )
            nc.sync.dma_start(out=rs_d[:], in_=rs[:])

    nc.compile()
    _CACHE["nc"] = nc
    return nc


def _prep_inputs(z_i, z_j):
    f8 = ml_dtypes.float8_e4m3
    zin = z_i / np.sqrt(np.sum(z_i * z_i, axis=1, keepdims=True))
    zjn = z_j / np.sqrt(np.sum(z_j * z_j, axis=1, keepdims=True))
    posn = np.sum(zin * zjn, axis=1, dtype=np.float64) / TEMP      # [4096]

    q8 = [(SC * zjn).astype(f8), (SC * zin).astype(f8)]
    # emulate the device's own diagonal term: fp32 G_ii, ACT exp, bf16 round
    diag = []
    for b in q8:
        bq = b.astype(np.float64)
        g = np.sum(bq * bq, axis=1)                   # = 256 * |qn_i|^2
        x = (g.astype(np.float32) * np.float32(ASCALE)).astype(np.float64) - 1.0 / TEMP
        e = np.exp(x).astype(np.float32).astype(ml_dtypes.bfloat16)
        diag.append(e.astype(np.float64))

    in_maps = []
    for c in range(NCORES):
        v = c // (NCORES // 2)
        s = c % (NCORES // 2)
        b = q8[v]
        bT = b.T                                       # [256, 4096]
        anT = np.ascontiguousarray(
            bT.reshape(2, 128, NH, NCH, 512).transpose(2, 3, 1, 0, 4)
        )
        slab = b[s * RPC:(s + 1) * RPC]
        qnT = np.ascontiguousarray(slab.T.reshape(2, 128, RPC).transpose(1, 0, 2))
        in_maps.append({"anT": anT, "qnT": qnT})
    return in_maps, posn, diag


def kernel(z_i, z_j):
    z_i = np.asarray(z_i, dtype=np.float32)
    z_j = np.asarray(z_j, dtype=np.float32)

    from concourse.bass_utils import run_bass_kernel_spmd

    nc = _build_program()
    in_maps, posn, diag = _prep_inputs(z_i, z_j)

    res = run_bass_kernel_spmd(nc, in_maps, list(range(NCORES)))
    _CACHE["last_results"] = res

    rowsum = np.empty(2 * N, dtype=np.float64)
    for c in range(NCORES):
        slab = res.results[c]["rs"].astype(np.float64).T.reshape(-1)  # [1024]
        rowsum[c * RPC:(c + 1) * RPC] = slab

    posn_g = np.concatenate([posn, posn])
    diag_g = np.concatenate(diag)
    epos_g = np.exp(posn_g - 1.0 / TEMP)

    lse = 1.0 / TEMP + np.log(rowsum - diag_g + epos_g)
    loss = np.mean(lse - posn_g)
    return np.array(loss, dtype=np.float32)
